# revision 2
# baseline (speedup 1.0000x reference)
"""MoE grouped w8a8 block-quant GEMM + gated combine for 8 Trainium2 cores. v2.

Same math as the baseline kernel (see kernel.py docstring for the
combine-commutes-with-GEMM derivation), restructured around the measured
CoreSim cost model:
  - the gather path serializes at ~0.83 ns per 512B row, so the x table is
    stored as fp8 e3m4 (512B rows, half of bf16) with per-row scale
    normalization: u[src,k] = x[src,k] * (xsc[src,c]/max_c xsc[src,:]) / 16,
    and the gate matrix carries g * max_c(xsc) * 16 in bf16. The combine
    matmul is mixed-dtype: fp8e3 stationary x bf16 moving.
  - gather descriptor generation costs 1359 + 0.34/row on the Pool engine,
    so few large gather chunks; the gated-combine runs on uniform 1024-row
    sub-chunks (one PSUM bank each) decoupled from gather chunk sizes.
  - PE warm-up matmuls run from t=0 so the p-state ramp finishes before the
    first real combine.
  - G folds to a single [128, TPC] bf16 matrix: row r contributes at
    G[r % 128, r // 8] (token t = r//8 holds slots 8t..8t+7).
"""

import numpy as np
import ml_dtypes

T, TOPK, K, N, E, B = 16384, 8, 512, 512, 32, 128
ROWS = T * TOPK
NCORES = 8
EL = E // NCORES            # experts per core
RPC = ROWS // NCORES        # routed rows per core (16384)
TPC = T // NCORES           # tokens per core (2048)
SUB = 1024                  # combine sub-chunk rows (= 128 tokens)
NSUB = RPC // SUB           # 16 sub-chunks per core
NTB = TPC // 128            # 16 token blocks per core

# gather chunk sizes (rows); must be multiples of SUB and sum to RPC
GS = [1024] * 16
N_WARM = 11                 # PE warm-up matmuls (ap 512 each)

_cache = {}


def _build(gs=tuple(GS)):
    from contextlib import ExitStack
    import concourse.bacc as bacc
    import concourse.tile as tile
    from concourse import mybir

    dt = mybir.dt
    nc = bacc.Bacc("TRN2", target_bir_lowering=False, debug=False,
                   enable_asserts=False)

    assert sum(gs) == RPC and all(r % SUB == 0 for r in gs)

    xq32 = nc.dram_tensor("xq32", (T, K // 4), dt.int32, kind="ExternalInput")
    wq = nc.dram_tensor("wq", (EL, 4, 128, 512), dt.bfloat16, kind="ExternalInput")
    idxw = nc.dram_tensor("idxw", (128, RPC // 16), dt.int16, kind="ExternalInput")
    gmat = nc.dram_tensor("gmat", (128, TPC), dt.bfloat16, kind="ExternalInput")
    shared = nc.dram_tensor("shared", (TPC, N), dt.bfloat16, kind="ExternalInput")
    out = nc.dram_tensor("out", (TPC, N), dt.bfloat16, kind="ExternalOutput")

    OP = mybir.AluOpType

    with tile.TileContext(nc) as tc, ExitStack() as ctx:
        const = ctx.enter_context(tc.tile_pool(name="const", bufs=1))
        gat_p = ctx.enter_context(tc.tile_pool(name="gat", bufs=3))
        zsb_p = ctx.enter_context(tc.tile_pool(name="zsb", bufs=4))
        sh_p = ctx.enter_context(tc.tile_pool(name="shp", bufs=3))
        ob_p = ctx.enter_context(tc.tile_pool(name="obp", bufs=3))
        zps_p = ctx.enter_context(tc.tile_pool(name="zps", bufs=3, space="PSUM"))
        ops_p = ctx.enter_context(tc.tile_pool(name="ops", bufs=3, space="PSUM"))
        wps_p = ctx.enter_context(tc.tile_pool(name="wps", bufs=1, space="PSUM"))

        # --- PE warm-up: keep the array busy from t=0 so the p-state ramp
        # finishes before the first combine.  Matmul on a zeroed tile.
        wz = const.tile([128, 512], dt.bfloat16)
        nc.vector.memset(wz[:], 0.0)
        wps = wps_p.tile([128, 512], dt.float32)
        for i in range(N_WARM):
            nc.tensor.matmul(wps[:], wz[:, 0:128], wz[:], start=True, stop=True)
        # preload the ACT 'copy' function table during the idle start so the
        # first real evict doesn't pay LoadActFuncSet
        wcp = const.tile([128, 1], dt.bfloat16)
        nc.scalar.copy(wcp[:], wz[:, 0:1])

        # --- index table: first gather chunk's slice lands first
        idx = const.tile([128, RPC // 16], dt.int16)
        c0 = gs[0] // 16
        nc.sync.dma_start(idx[:, 0:c0], idxw.ap()[:, 0:c0])
        nc.sync.dma_start(idx[:, c0:], idxw.ap()[:, c0:])

        # --- gate matrix [128, TPC]; two pieces so the first tokens unblock
        Gall = const.tile([128, TPC], dt.bfloat16)
        nc.scalar.dma_start(Gall[:, 0:512], gmat.ap()[:, 0:512])
        nc.scalar.dma_start(Gall[:, 512:], gmat.ap()[:, 512:])

        # --- per-expert dequantized weights, JIT
        wdeq = [const.tile([128, 4 * 512], dt.bfloat16, name=f"wdeq{e}")
                for e in range(EL)]

        def load_weights(e, eng=None):
            (eng or nc.sync).dma_start(
                wdeq[e][:].rearrange("p (c n) -> p c n", c=4),
                wq.ap()[e].rearrange("c p n -> p c n"))

        gtiles = []          # (tile, row0, nblocks), appended as issued
        state = {}

        def main_gemm(pend):
            # deferred: z for token block tb is ready in SBUF
            zsb, tb = pend
            e = tb // (NTB // EL)
            if tb % 2 == 0:
                sh2 = sh_p.tile([128, 2 * 512], dt.bfloat16)
                eng = nc.scalar if (tb // 2) % 2 == 0 else nc.sync
                eng.dma_start(
                    sh2[:].rearrange("p (q n) -> p q n", q=2),
                    shared.ap()[tb * 128:(tb + 2) * 128, :]
                    .rearrange("(q p) n -> p q n", q=2))
                ob2 = ob_p.tile([128, 2 * 512], dt.bfloat16)
                state["sh2"] = sh2
                state["ob2"] = ob2
            sh2, ob2 = state["sh2"], state["ob2"]
            ops = ops_p.tile([128, 512], dt.float32)
            wv = wdeq[e][:].rearrange("p (c n) -> p c n", c=4)
            for c in range(4):
                nc.tensor.matmul(
                    ops[:], zsb[:, c * 128:(c + 1) * 128],
                    wv[:, c, :], start=(c == 0), stop=(c == 3))
            half = tb % 2
            nc.vector.tensor_tensor(ob2[:, half * 512:(half + 1) * 512],
                                    ops[:], sh2[:, half * 512:(half + 1) * 512],
                                    OP.add)
            if tb == NTB - 1:
                nc.sync.dma_start(
                    out.ap()[(tb - 1) * 128:tb * 128, :], ob2[:, 0:512])
                nc.sync.dma_start(
                    out.ap()[tb * 128:(tb + 1) * 128, :], ob2[:, 512:1024])
            elif half == 1:
                eng = nc.sync if (tb // 2) % 2 == 0 else nc.scalar
                eng.dma_start(
                    out.ap()[(tb - 1) * 128:(tb + 1) * 128, :]
                    .rearrange("(q p) n -> p q n", q=2),
                    ob2[:].rearrange("p (q n) -> p q n", q=2))

        # --- main pipeline: issue gathers JIT, combine per 1024-row sub-chunk
        # expert 3's weights ride the Pool/SWDGE path during the idle window
        # before the index table lands (keeps HWDGE under the 360 GB/s pool)
        load_weights(3, nc.gpsimd)
        load_weights(0)
        max_nb = max(gs) // 128
        pending = None
        gi = 0               # next gather chunk to issue
        issued_rows = 0
        for tb in range(NTB):
            # issue gather chunks until sub-chunk tb's rows are covered
            while issued_rows < (tb + 1) * SUB and gi < len(gs):
                R = gs[gi]
                nb = R // 128
                roff = issued_rows
                xg = gat_p.tile([128, max_nb * 128], dt.int32, name="xg")
                gtiles.append((xg, roff, nb))
                nc.gpsimd.dma_gather(
                    xg[:, 0:nb * 128].rearrange("p (b i) -> p b i", b=nb),
                    xq32.ap(),
                    idx[:, roff // 16:(roff + R) // 16],
                    R, R, K // 4,
                    transpose=False, single_packet=False)
                issued_rows += R
                gi += 1
                if gi == 2:
                    load_weights(1)
                    load_weights(2)
            # locate the gather tile slice for this sub-chunk
            row0 = tb * SUB
            for xg, roff, nb in gtiles:
                if roff <= row0 < roff + nb * 128:
                    b0 = (row0 - roff) // 128
                    break
            Xb = xg[:, 0:nb * 128].bitcast(dt.float8e3).rearrange(
                "p (b k) -> p b k", b=nb)
            # combine: z[k, 16b + p//8] for 8 row blocks
            zps = zps_p.tile([128, 4 * 128], dt.float32)
            for b in range(8):
                gcol = tb * 128 + b * 16
                for c in range(4):
                    nc.tensor.matmul(
                        zps[:, c * 128 + 16 * b: c * 128 + 16 * b + 16],
                        Xb[:, b0 + b, c * 128:(c + 1) * 128],
                        Gall[:, gcol:gcol + 16],
                        start=True, stop=True)
            if pending is not None:
                main_gemm(pending)
            zsb = zsb_p.tile([128, 4 * 128], dt.bfloat16)
            if tb % 2 == 0 or tb >= NTB - 2:
                # endgame evicts on ACT: its queue is empty by then, DVE's isn't
                nc.scalar.copy(zsb[:], zps[:])
            else:
                nc.vector.tensor_copy(zsb[:], zps[:])
            pending = (zsb, tb)
        main_gemm(pending)

    nc.compile()
    return nc


def _prep_inputs(input, weight, top_k_gates, token_indices, src_to_dst,
                 token_count, shared_output, weight_scale, input_scale):
    bf16 = ml_dtypes.bfloat16
    e3 = ml_dtypes.float8_e3m4
    x = np.asarray(input, dtype=np.int8)
    w = np.asarray(weight, dtype=np.int8)
    tkg = np.asarray(top_k_gates, dtype=np.float32)
    ti = np.asarray(token_indices, dtype=np.int32)
    s2d = np.asarray(src_to_dst, dtype=np.int32)
    sho = np.asarray(shared_output).astype(bf16)
    wsc = np.asarray(weight_scale, dtype=np.float32)
    xsc = np.asarray(input_scale, dtype=np.float32)

    # fp8 e3m4 table with per-row scale normalization
    S = xsc.max(axis=1)                                   # [T]
    ratio = xsc / S[:, None]                              # [T,4] in (0,1]
    uq = (x.astype(np.float32).reshape(T, 4, B)
          * (ratio[:, :, None] / 16.0)).astype(e3)        # [T,4,128] fp8
    xq32 = np.ascontiguousarray(uq).reshape(T, K).view(np.int32)  # [T,128]

    # dequantized weights [E, 4(c), 128(p), 512(n)] bf16
    wdeqh = (w.astype(np.float32)
             * np.repeat(np.repeat(wsc, B, axis=1), B, axis=2)
             ).astype(bf16).reshape(E, 4, 128, 512)

    # normalized, drop-masked gates; fold S[src] * 16 back in
    gn = tkg / np.clip(tkg.sum(axis=-1, keepdims=True), 1e-12, None)
    gn = np.where(s2d == -1, 0.0, gn)                    # [T, TOPK]
    grows = gn.reshape(ROWS)
    coef = (grows * S[ti] * 16.0).astype(bf16)           # [ROWS]

    in_maps = []
    for cid in range(NCORES):
        e0 = cid * EL
        r0 = cid * RPC
        tl = ti[r0:r0 + RPC].astype(np.int16)
        idx16 = np.ascontiguousarray(tl.reshape(-1, 16).T)      # [16, RPC/16]
        idxw = np.tile(idx16, (8, 1))                            # [128, RPC/16]
        # G[p, t] = coef[8t + p%8] restricted to this core's tokens
        cc = coef[r0:r0 + RPC]                                   # [RPC]
        gm = np.zeros((128, TPC), bf16)
        r = np.arange(RPC)
        gm[r % 128, r // 8] = cc
        t0 = cid * TPC
        in_maps.append({
            "xq32": xq32,
            "wq": np.ascontiguousarray(wdeqh[e0:e0 + EL]),
            "idxw": idxw,
            "gmat": gm,
            "shared": np.ascontiguousarray(sho[t0:t0 + TPC]),
        })
    return in_maps


def kernel(**inputs):
    from concourse import bass_utils
    if "nc" not in _cache:
        _cache["nc"] = _build()
    nc = _cache["nc"]
    in_maps = _prep_inputs(**inputs)
    import os
    res = bass_utils.run_bass_kernel_spmd(
        nc, in_maps, core_ids=list(range(NCORES)),
        trace=os.environ.get("BASS_TRACE") == "1")
    _cache["last_results"] = res
    out = np.concatenate([res.results[c]["out"] for c in range(NCORES)], axis=0)
    return out


# revision 3
# speedup vs baseline: 1.0326x; 1.0326x over previous
"""MoE grouped w8a8 block-quant GEMM + gated combine for 8 Trainium2 cores. v2.

Same math as the baseline kernel (see kernel.py docstring for the
combine-commutes-with-GEMM derivation), restructured around the measured
CoreSim cost model:
  - the gather path serializes at ~0.83 ns per 512B row, so the x table is
    stored as fp8 e3m4 (512B rows, half of bf16) with per-row scale
    normalization: u[src,k] = x[src,k] * (xsc[src,c]/max_c xsc[src,:]) / 16,
    and the gate matrix carries g * max_c(xsc) * 16 in bf16. The combine
    matmul is mixed-dtype: fp8e3 stationary x bf16 moving.
  - gather descriptor generation costs 1359 + 0.34/row on the Pool engine,
    so few large gather chunks; the gated-combine runs on uniform 1024-row
    sub-chunks (one PSUM bank each) decoupled from gather chunk sizes.
  - PE warm-up matmuls run from t=0 so the p-state ramp finishes before the
    first real combine.
  - G folds to a single [128, TPC] bf16 matrix: row r contributes at
    G[r % 128, r // 8] (token t = r//8 holds slots 8t..8t+7).
"""

import numpy as np
import ml_dtypes

T, TOPK, K, N, E, B = 16384, 8, 512, 512, 32, 128
ROWS = T * TOPK
NCORES = 8
EL = E // NCORES            # experts per core
RPC = ROWS // NCORES        # routed rows per core (16384)
TPC = T // NCORES           # tokens per core (2048)
SUB = 1024                  # combine sub-chunk rows (= 128 tokens)
NSUB = RPC // SUB           # 16 sub-chunks per core
NTB = TPC // 128            # 16 token blocks per core

# gather chunk sizes (rows); must be multiples of SUB and sum to RPC
GS = [1024] * 16
N_WARM = 10                 # PE warm-up matmuls (ap 512 each)

_cache = {}


def _build(gs=tuple(GS)):
    from contextlib import ExitStack
    import concourse.bacc as bacc
    import concourse.tile as tile
    from concourse import mybir

    dt = mybir.dt
    nc = bacc.Bacc("TRN2", target_bir_lowering=False, debug=False,
                   enable_asserts=False)

    assert sum(gs) == RPC and all(r % SUB == 0 for r in gs)

    xq32 = nc.dram_tensor("xq32", (T, K // 4), dt.int32, kind="ExternalInput")
    wq = nc.dram_tensor("wq", (EL, 4, 128, 512), dt.bfloat16, kind="ExternalInput")
    idxw = nc.dram_tensor("idxw", (128, RPC // 16), dt.int16, kind="ExternalInput")
    gmat = nc.dram_tensor("gmat", (128, TPC), dt.bfloat16, kind="ExternalInput")
    shared = nc.dram_tensor("shared", (TPC, N), dt.bfloat16, kind="ExternalInput")
    out = nc.dram_tensor("out", (TPC, N), dt.bfloat16, kind="ExternalOutput")

    OP = mybir.AluOpType

    with tile.TileContext(nc) as tc, ExitStack() as ctx:
        const = ctx.enter_context(tc.tile_pool(name="const", bufs=1))
        gat_p = ctx.enter_context(tc.tile_pool(name="gat", bufs=5))
        zsb_p = ctx.enter_context(tc.tile_pool(name="zsb", bufs=4))
        sh_p = ctx.enter_context(tc.tile_pool(name="shp", bufs=3))
        ob_p = ctx.enter_context(tc.tile_pool(name="obp", bufs=3))
        zps_p = ctx.enter_context(tc.tile_pool(name="zps", bufs=4, space="PSUM"))
        ops_p = ctx.enter_context(tc.tile_pool(name="ops", bufs=3, space="PSUM"))
        wps_p = ctx.enter_context(tc.tile_pool(name="wps", bufs=1, space="PSUM"))

        # --- PE warm-up: keep the array busy from t=0 so the p-state ramp
        # finishes before the first combine.  Matmul on a zeroed tile.
        wz = const.tile([128, 512], dt.bfloat16)
        nc.vector.memset(wz[:], 0.0)
        wps = wps_p.tile([128, 512], dt.float32)
        state_wps = wps
        for i in range(N_WARM):
            nc.tensor.matmul(wps[:], wz[:, 0:128], wz[:], start=True, stop=True)
        # preload the ACT 'copy' function table during the idle start so the
        # first real evict doesn't pay LoadActFuncSet
        wcp = const.tile([128, 1], dt.bfloat16)
        nc.scalar.copy(wcp[:], wz[:, 0:1])

        # --- index table: first gather chunk's slice lands first
        idx = const.tile([128, RPC // 16], dt.int16)
        c0 = gs[0] // 16
        nc.sync.dma_start(idx[:, 0:c0], idxw.ap()[:, 0:c0])
        nc.sync.dma_start(idx[:, c0:], idxw.ap()[:, c0:])

        # --- gate matrix [128, TPC]; two pieces so the first tokens unblock
        Gall = const.tile([128, TPC], dt.bfloat16)
        nc.scalar.dma_start(Gall[:, 0:512], gmat.ap()[:, 0:512])
        nc.scalar.dma_start(Gall[:, 512:], gmat.ap()[:, 512:])

        # --- per-expert dequantized weights, JIT
        wdeq = [const.tile([128, 4 * 512], dt.bfloat16, name=f"wdeq{e}")
                for e in range(EL)]

        def load_weights(e, eng=None):
            (eng or nc.sync).dma_start(
                wdeq[e][:].rearrange("p (c n) -> p c n", c=4),
                wq.ap()[e].rearrange("c p n -> p c n"))

        gtiles = []          # (tile, row0, nblocks), appended as issued
        state = {}

        def main_gemm(pend):
            # deferred: z for token block tb is ready in SBUF
            zsb, tb = pend
            e = tb // (NTB // EL)
            if tb % 2 == 0:
                sh2 = sh_p.tile([128, 2 * 512], dt.bfloat16)
                eng = nc.scalar if (tb // 2) % 2 == 0 else nc.sync
                eng.dma_start(
                    sh2[:].rearrange("p (q n) -> p q n", q=2),
                    shared.ap()[tb * 128:(tb + 2) * 128, :]
                    .rearrange("(q p) n -> p q n", q=2))
                ob2 = ob_p.tile([128, 2 * 512], dt.bfloat16)
                state["sh2"] = sh2
                state["ob2"] = ob2
            sh2, ob2 = state["sh2"], state["ob2"]
            wv = wdeq[e][:].rearrange("p (c n) -> p c n", c=4)
            half = tb % 2
            ops = ops_p.tile([128, 512], dt.float32)
            for c in range(4):
                nc.tensor.matmul(
                    ops[:], zsb[:, c * 128:(c + 1) * 128],
                    wv[:, c, :], start=(c == 0), stop=(c == 3))
            nc.vector.tensor_tensor(ob2[:, half * 512:(half + 1) * 512],
                                    ops[:], sh2[:, half * 512:(half + 1) * 512],
                                    OP.add)
            if tb == NTB - 1:
                # final stores split across both HWDGE queues so the two
                # dge+sem chains run in parallel
                nc.sync.dma_start(
                    out.ap()[(tb - 1) * 128:tb * 128, :], ob2[:, 0:512])
                nc.sync.dma_start(
                    out.ap()[tb * 128:(tb + 1) * 128, 0:256], ob2[:, 512:768])
                nc.scalar.dma_start(
                    out.ap()[tb * 128:(tb + 1) * 128, 256:512], ob2[:, 768:1024])
            elif half == 1:
                eng = nc.sync if (tb // 2) % 2 == 0 else nc.scalar
                eng.dma_start(
                    out.ap()[(tb - 1) * 128:(tb + 1) * 128, :]
                    .rearrange("(q p) n -> p q n", q=2),
                    ob2[:].rearrange("p (q n) -> p q n", q=2))

        # --- main pipeline: issue gathers JIT, combine per 1024-row sub-chunk
        # expert 3's weights ride the Pool/SWDGE path during the idle window
        # before the index table lands (keeps HWDGE under the 360 GB/s pool)
        load_weights(3, nc.gpsimd)
        load_weights(0)
        max_nb = max(gs) // 128
        pending = None
        gi = 0               # next gather chunk to issue
        issued_rows = 0
        for tb in range(NTB):
            # issue gather chunks until sub-chunk tb's rows are covered
            while issued_rows < (tb + 1) * SUB and gi < len(gs):
                R = gs[gi]
                nb = R // 128
                roff = issued_rows
                xg = gat_p.tile([128, max_nb * 128], dt.int32, name="xg")
                gtiles.append((xg, roff, nb))
                nc.gpsimd.dma_gather(
                    xg[:, 0:nb * 128].rearrange("p (b i) -> p b i", b=nb),
                    xq32.ap(),
                    idx[:, roff // 16:(roff + R) // 16],
                    R, R, K // 4,
                    transpose=False, single_packet=False)
                issued_rows += R
                gi += 1
                if gi == 2:
                    load_weights(1)
                    load_weights(2)
            # locate the gather tile slice for this sub-chunk
            row0 = tb * SUB
            for xg, roff, nb in gtiles:
                if roff <= row0 < roff + nb * 128:
                    b0 = (row0 - roff) // 128
                    break
            Xb = xg[:, 0:nb * 128].bitcast(dt.float8e3).rearrange(
                "p (b k) -> p b k", b=nb)
            # combine: z[k, 16b + p//8] for 8 row blocks
            zps = zps_p.tile([128, 4 * 128], dt.float32)
            for b in range(8):
                gcol = tb * 128 + b * 16
                for c in range(4):
                    nc.tensor.matmul(
                        zps[:, c * 128 + 16 * b: c * 128 + 16 * b + 16],
                        Xb[:, b0 + b, c * 128:(c + 1) * 128],
                        Gall[:, gcol:gcol + 16],
                        start=True, stop=True)
            if pending is not None:
                main_gemm(pending)
            zsb = zsb_p.tile([128, 4 * 128], dt.bfloat16)
            if tb % 2 == 0 or tb >= NTB - 2:
                # endgame evicts on ACT: its queue is empty by then, DVE's isn't
                nc.scalar.copy(zsb[:], zps[:])
            else:
                nc.vector.tensor_copy(zsb[:], zps[:])
            pending = (zsb, tb)
        main_gemm(pending)

    nc.compile()
    return nc


def _prep_inputs(input, weight, top_k_gates, token_indices, src_to_dst,
                 token_count, shared_output, weight_scale, input_scale):
    bf16 = ml_dtypes.bfloat16
    e3 = ml_dtypes.float8_e3m4
    x = np.asarray(input, dtype=np.int8)
    w = np.asarray(weight, dtype=np.int8)
    tkg = np.asarray(top_k_gates, dtype=np.float32)
    ti = np.asarray(token_indices, dtype=np.int32)
    s2d = np.asarray(src_to_dst, dtype=np.int32)
    sho = np.asarray(shared_output).astype(bf16)
    wsc = np.asarray(weight_scale, dtype=np.float32)
    xsc = np.asarray(input_scale, dtype=np.float32)

    # fp8 e3m4 table with per-row scale normalization
    S = xsc.max(axis=1)                                   # [T]
    ratio = xsc / S[:, None]                              # [T,4] in (0,1]
    uq = (x.astype(np.float32).reshape(T, 4, B)
          * (ratio[:, :, None] / 16.0)).astype(e3)        # [T,4,128] fp8
    xq32 = np.ascontiguousarray(uq).reshape(T, K).view(np.int32)  # [T,128]

    # dequantized weights [E, 4(c), 128(p), 512(n)] bf16
    wdeqh = (w.astype(np.float32)
             * np.repeat(np.repeat(wsc, B, axis=1), B, axis=2)
             ).astype(bf16).reshape(E, 4, 128, 512)

    # normalized, drop-masked gates; fold S[src] * 16 back in
    gn = tkg / np.clip(tkg.sum(axis=-1, keepdims=True), 1e-12, None)
    gn = np.where(s2d == -1, 0.0, gn)                    # [T, TOPK]
    grows = gn.reshape(ROWS)
    coef = (grows * S[ti] * 16.0).astype(bf16)           # [ROWS]

    in_maps = []
    for cid in range(NCORES):
        e0 = cid * EL
        r0 = cid * RPC
        tl = ti[r0:r0 + RPC].astype(np.int16)
        idx16 = np.ascontiguousarray(tl.reshape(-1, 16).T)      # [16, RPC/16]
        idxw = np.tile(idx16, (8, 1))                            # [128, RPC/16]
        # G[p, t] = coef[8t + p%8] restricted to this core's tokens
        cc = coef[r0:r0 + RPC]                                   # [RPC]
        gm = np.zeros((128, TPC), bf16)
        r = np.arange(RPC)
        gm[r % 128, r // 8] = cc
        t0 = cid * TPC
        in_maps.append({
            "xq32": xq32,
            "wq": np.ascontiguousarray(wdeqh[e0:e0 + EL]),
            "idxw": idxw,
            "gmat": gm,
            "shared": np.ascontiguousarray(sho[t0:t0 + TPC]),
        })
    return in_maps


def kernel(**inputs):
    from concourse import bass_utils
    if "nc" not in _cache:
        _cache["nc"] = _build()
    nc = _cache["nc"]
    in_maps = _prep_inputs(**inputs)
    import os
    res = bass_utils.run_bass_kernel_spmd(
        nc, in_maps, core_ids=list(range(NCORES)),
        trace=os.environ.get("BASS_TRACE") == "1")
    _cache["last_results"] = res
    out = np.concatenate([res.results[c]["out"] for c in range(NCORES)], axis=0)
    return out


# revision 5
# speedup vs baseline: 1.0450x; 1.0121x over previous
"""MoE grouped w8a8 block-quant GEMM + gated combine for 8 Trainium2 cores. v2.

Same math as the baseline kernel (see kernel.py docstring for the
combine-commutes-with-GEMM derivation), restructured around the measured
CoreSim cost model:
  - the gather path serializes at ~0.83 ns per 512B row, so the x table is
    stored as fp8 e3m4 (512B rows, half of bf16) with per-row scale
    normalization: u[src,k] = x[src,k] * (xsc[src,c]/max_c xsc[src,:]) / 16,
    and the gate matrix carries g * max_c(xsc) * 16 in bf16. The combine
    matmul is mixed-dtype: fp8e3 stationary x bf16 moving.
  - gather descriptor generation costs 1359 + 0.34/row on the Pool engine,
    so few large gather chunks; the gated-combine runs on uniform 1024-row
    sub-chunks (one PSUM bank each) decoupled from gather chunk sizes.
  - PE warm-up matmuls run from t=0 so the p-state ramp finishes before the
    first real combine.
  - G folds to a single [128, TPC] bf16 matrix: row r contributes at
    G[r % 128, r // 8] (token t = r//8 holds slots 8t..8t+7).
"""

import numpy as np
import ml_dtypes

T, TOPK, K, N, E, B = 16384, 8, 512, 512, 32, 128
ROWS = T * TOPK
NCORES = 8
EL = E // NCORES            # experts per core
RPC = ROWS // NCORES        # routed rows per core (16384)
TPC = T // NCORES           # tokens per core (2048)
SUB = 1024                  # combine sub-chunk rows (= 128 tokens)
NSUB = RPC // SUB           # 16 sub-chunks per core
NTB = TPC // 128            # 16 token blocks per core

# gather chunk sizes (rows); must be multiples of SUB and sum to RPC
GS = [1024] * 16

_cache = {}


def _build(gs=tuple(GS)):
    from contextlib import ExitStack
    import concourse.bacc as bacc
    import concourse.tile as tile
    from concourse import mybir

    dt = mybir.dt
    nc = bacc.Bacc("TRN2", target_bir_lowering=False, debug=False,
                   enable_asserts=False)

    assert sum(gs) == RPC and all(r % SUB == 0 for r in gs)

    xq32 = nc.dram_tensor("xq32", (T, K // 4), dt.int32, kind="ExternalInput")
    wq = nc.dram_tensor("wq", (EL, 4, 128, 512), dt.bfloat16, kind="ExternalInput")
    idxw = nc.dram_tensor("idxw", (128, RPC // 16), dt.int16, kind="ExternalInput")
    gmat = nc.dram_tensor("gmat", (128, TPC), dt.bfloat16, kind="ExternalInput")
    shared = nc.dram_tensor("shared", (TPC, N), dt.bfloat16, kind="ExternalInput")
    out = nc.dram_tensor("out", (TPC, N), dt.bfloat16, kind="ExternalOutput")

    OP = mybir.AluOpType

    with tile.TileContext(nc) as tc, ExitStack() as ctx:
        const = ctx.enter_context(tc.tile_pool(name="const", bufs=1))
        gat_p = ctx.enter_context(tc.tile_pool(name="gat", bufs=5))
        zsb_p = ctx.enter_context(tc.tile_pool(name="zsb", bufs=4))
        sh_p = ctx.enter_context(tc.tile_pool(name="shp", bufs=3))
        ob_p = ctx.enter_context(tc.tile_pool(name="obp", bufs=3))
        zps_p = ctx.enter_context(tc.tile_pool(name="zps", bufs=6, space="PSUM"))
        ops_p = ctx.enter_context(tc.tile_pool(name="ops", bufs=2, space="PSUM"))
        # preload the ACT 'copy' function table during the idle start so the
        # first real evict doesn't pay LoadActFuncSet
        wz = const.tile([128, 1], dt.bfloat16)
        nc.vector.memset(wz[:], 0.0)
        wcp = const.tile([128, 1], dt.bfloat16)
        nc.scalar.copy(wcp[:], wz[:])

        # --- index table: first gather chunk's slice lands first
        idx = const.tile([128, RPC // 16], dt.int16)
        c0 = gs[0] // 16
        nc.sync.dma_start(idx[:, 0:c0], idxw.ap()[:, 0:c0])
        nc.sync.dma_start(idx[:, c0:], idxw.ap()[:, c0:])

        # --- gate matrix [128, TPC]; two pieces so the first tokens unblock
        Gall = const.tile([128, TPC], dt.bfloat16)
        nc.scalar.dma_start(Gall[:, 0:512], gmat.ap()[:, 0:512])
        nc.scalar.dma_start(Gall[:, 512:], gmat.ap()[:, 512:])

        # --- per-expert dequantized weights, JIT
        wdeq = [const.tile([128, 4 * 512], dt.bfloat16, name=f"wdeq{e}")
                for e in range(EL)]

        def load_weights(e, eng=None):
            (eng or nc.sync).dma_start(
                wdeq[e][:].rearrange("p (c n) -> p c n", c=4),
                wq.ap()[e].rearrange("c p n -> p c n"))

        gtiles = []          # (tile, row0, nblocks), appended as issued
        state = {}

        def main_gemm(pend):
            # deferred: z for token block tb is ready in SBUF
            zsb, tb = pend
            e = tb // (NTB // EL)
            if tb % 2 == 0:
                sh2 = sh_p.tile([128, 2 * 512], dt.bfloat16)
                eng = nc.scalar if (tb // 2) % 2 == 0 else nc.sync
                eng.dma_start(
                    sh2[:].rearrange("p (q n) -> p q n", q=2),
                    shared.ap()[tb * 128:(tb + 2) * 128, :]
                    .rearrange("(q p) n -> p q n", q=2))
                ob2 = ob_p.tile([128, 2 * 512], dt.bfloat16)
                state["sh2"] = sh2
                state["ob2"] = ob2
            sh2, ob2 = state["sh2"], state["ob2"]
            wv = wdeq[e][:].rearrange("p (c n) -> p c n", c=4)
            half = tb % 2
            ops = ops_p.tile([128, 512], dt.float32)
            for c in range(4):
                nc.tensor.matmul(
                    ops[:], zsb[:, c * 128:(c + 1) * 128],
                    wv[:, c, :], start=(c == 0), stop=(c == 3))
            nc.vector.tensor_tensor(ob2[:, half * 512:(half + 1) * 512],
                                    ops[:], sh2[:, half * 512:(half + 1) * 512],
                                    OP.add)
            if tb == NTB - 1:
                # final stores split across both HWDGE queues so the two
                # dge+sem chains run in parallel
                nc.sync.dma_start(
                    out.ap()[(tb - 1) * 128:tb * 128, :], ob2[:, 0:512])
                nc.sync.dma_start(
                    out.ap()[tb * 128:(tb + 1) * 128, 0:256], ob2[:, 512:768])
                nc.scalar.dma_start(
                    out.ap()[tb * 128:(tb + 1) * 128, 256:512], ob2[:, 768:1024])
            elif half == 1:
                eng = nc.sync if (tb // 2) % 2 == 0 else nc.scalar
                eng.dma_start(
                    out.ap()[(tb - 1) * 128:(tb + 1) * 128, :]
                    .rearrange("(q p) n -> p q n", q=2),
                    ob2[:].rearrange("p (q n) -> p q n", q=2))

        # --- main pipeline: issue gathers JIT, combine per 1024-row sub-chunk
        # expert 3's weights ride the Pool/SWDGE path during the idle window
        # before the index table lands (keeps HWDGE under the 360 GB/s pool)
        load_weights(3, nc.gpsimd)
        load_weights(0)
        max_nb = max(gs) // 128
        pending = None
        gi = 0               # next gather chunk to issue
        issued_rows = 0
        for tb in range(NTB):
            # issue gather chunks until sub-chunk tb's rows are covered
            while issued_rows < (tb + 1) * SUB and gi < len(gs):
                R = gs[gi]
                nb = R // 128
                roff = issued_rows
                xg = gat_p.tile([128, max_nb * 128], dt.int32, name="xg")
                gtiles.append((xg, roff, nb))
                nc.gpsimd.dma_gather(
                    xg[:, 0:nb * 128].rearrange("p (b i) -> p b i", b=nb),
                    xq32.ap(),
                    idx[:, roff // 16:(roff + R) // 16],
                    R, R, K // 4,
                    transpose=False, single_packet=False)
                issued_rows += R
                gi += 1
                if gi == 2:
                    load_weights(1)
                    load_weights(2)
            # locate the gather tile slice for this sub-chunk
            row0 = tb * SUB
            for xg, roff, nb in gtiles:
                if roff <= row0 < roff + nb * 128:
                    b0 = (row0 - roff) // 128
                    break
            Xb = xg[:, 0:nb * 128].bitcast(dt.float8e3).rearrange(
                "p (b k) -> p b k", b=nb)
            # combine: z[k, 16b + p//8] for 8 row blocks
            zps = zps_p.tile([128, 4 * 128], dt.float32)
            for b in range(8):
                gcol = tb * 128 + b * 16
                for c in range(4):
                    nc.tensor.matmul(
                        zps[:, c * 128 + 16 * b: c * 128 + 16 * b + 16],
                        Xb[:, b0 + b, c * 128:(c + 1) * 128],
                        Gall[:, gcol:gcol + 16],
                        start=True, stop=True)
            if pending is not None:
                main_gemm(pending)
            zsb = zsb_p.tile([128, 4 * 128], dt.bfloat16)
            if tb % 2 == 0 or tb >= NTB - 2:
                # endgame evicts on ACT: its queue is empty by then, DVE's isn't
                nc.scalar.copy(zsb[:], zps[:])
            else:
                nc.vector.tensor_copy(zsb[:], zps[:])
            pending = (zsb, tb)
        main_gemm(pending)

    nc.compile()
    return nc


def _prep_inputs(input, weight, top_k_gates, token_indices, src_to_dst,
                 token_count, shared_output, weight_scale, input_scale):
    bf16 = ml_dtypes.bfloat16
    e3 = ml_dtypes.float8_e3m4
    x = np.asarray(input, dtype=np.int8)
    w = np.asarray(weight, dtype=np.int8)
    tkg = np.asarray(top_k_gates, dtype=np.float32)
    ti = np.asarray(token_indices, dtype=np.int32)
    s2d = np.asarray(src_to_dst, dtype=np.int32)
    sho = np.asarray(shared_output).astype(bf16)
    wsc = np.asarray(weight_scale, dtype=np.float32)
    xsc = np.asarray(input_scale, dtype=np.float32)

    # fp8 e3m4 table with per-row scale normalization
    S = xsc.max(axis=1)                                   # [T]
    ratio = xsc / S[:, None]                              # [T,4] in (0,1]
    uq = (x.astype(np.float32).reshape(T, 4, B)
          * (ratio[:, :, None] / 16.0)).astype(e3)        # [T,4,128] fp8
    xq32 = np.ascontiguousarray(uq).reshape(T, K).view(np.int32)  # [T,128]

    # dequantized weights [E, 4(c), 128(p), 512(n)] bf16
    wdeqh = (w.astype(np.float32)
             * np.repeat(np.repeat(wsc, B, axis=1), B, axis=2)
             ).astype(bf16).reshape(E, 4, 128, 512)

    # normalized, drop-masked gates; fold S[src] * 16 back in
    gn = tkg / np.clip(tkg.sum(axis=-1, keepdims=True), 1e-12, None)
    gn = np.where(s2d == -1, 0.0, gn)                    # [T, TOPK]
    grows = gn.reshape(ROWS)
    coef = (grows * S[ti] * 16.0).astype(bf16)           # [ROWS]

    in_maps = []
    for cid in range(NCORES):
        e0 = cid * EL
        r0 = cid * RPC
        tl = ti[r0:r0 + RPC].astype(np.int16)
        idx16 = np.ascontiguousarray(tl.reshape(-1, 16).T)      # [16, RPC/16]
        idxw = np.tile(idx16, (8, 1))                            # [128, RPC/16]
        # G[p, t] = coef[8t + p%8] restricted to this core's tokens
        cc = coef[r0:r0 + RPC]                                   # [RPC]
        gm = np.zeros((128, TPC), bf16)
        r = np.arange(RPC)
        gm[r % 128, r // 8] = cc
        t0 = cid * TPC
        in_maps.append({
            "xq32": xq32,
            "wq": np.ascontiguousarray(wdeqh[e0:e0 + EL]),
            "idxw": idxw,
            "gmat": gm,
            "shared": np.ascontiguousarray(sho[t0:t0 + TPC]),
        })
    return in_maps


def kernel(**inputs):
    from concourse import bass_utils
    if "nc" not in _cache:
        _cache["nc"] = _build()
    nc = _cache["nc"]
    in_maps = _prep_inputs(**inputs)
    import os
    res = bass_utils.run_bass_kernel_spmd(
        nc, in_maps, core_ids=list(range(NCORES)),
        trace=os.environ.get("BASS_TRACE") == "1")
    _cache["last_results"] = res
    out = np.concatenate([res.results[c]["out"] for c in range(NCORES)], axis=0)
    return out
